# revision 12
# baseline (speedup 1.0000x reference)
"""Trainium2 Bass kernel for nn_LSTMFeatureExtractor.

Math (per reference):
  x_proj = einsum('bsf,fg->sbg', obs, Wi)
  (c,h) LSTM recurrence over S steps with gates (i,f,g,o); out = relu(h @ Wd + bd)

Strategy:
  - Data parallel: batch 2048 split across 8 cores (256 each); weights replicated.
  - Per core, everything is computed in "transposed" layout: gatesT [4H, B] so that
    batch is the matmul free dim (N=256) and weights are the stationary operand.
  - Gate order is permuted to [i, f, o, g] per h-tile so one sigmoid covers a
    contiguous [128, 768] PSUM span and one tanh covers [128, 256].
  - bh is folded in as a 65th "ones" feature of obs (host-side augmentation);
    bd is applied as the per-partition bias of the final Relu activation.
  - bf16 matmul operands + elementwise, fp32 PSUM accumulation (verified
    numerically: rel l2 error vs fp32 reference ~5e-3).
  - x slices are transposed on-chip with PE transposes (batched 4 steps at a
    time through one PSUM bank), since DMA cannot efficiently load f-major.
"""

import numpy as np
import ml_dtypes

import concourse.bass as bass
import concourse.tile as tile
from concourse import mybir
from concourse.bass_utils import run_bass_kernel_spmd
from concourse.vector_clock import ScopedClock

BF16 = ml_dtypes.bfloat16

B, S, F = 2048, 256, 64
H, D = 256, 128
G = 4 * H           # 1024
NCORES = 8
BL = B // NCORES    # 256 batch rows per core
FA = F + 1          # augmented feature dim (ones column carries bh)
CH = 8              # obs DMA chunk, in steps
TB = 4              # PE-transpose batch, in steps

AFT = mybir.ActivationFunctionType

# ---------------------------------------------------------------------------
# Workaround for this walrus build: a CTRL (Drain) instruction only accepts a
# single sync-wait command; Tile's tail drain attaches one wait per live
# semaphore. Split them across multiple drain instructions (1 wait each).
_PATCHED = False


def _install_drain_patch():
    global _PATCHED
    if _PATCHED:
        return
    _PATCHED = True

    def _drain_and_barrier(self, tick_clock, wait_clock):
        nc = self.nc
        drain_inst = nc.sync.drain()
        wait_clock.add_sem_waits(
            drain_inst.ins, ScopedClock({None: tick_clock.global_clock})
        )
        si = drain_inst.ins.sync_info
        if si is not None and si.on_wait and len(si.on_wait) > 1:
            waits = list(si.on_wait)
            si.on_wait = waits[:1]
            for w in waits[1:]:
                d2 = nc.sync.drain()
                si2 = d2.ins.sync_info
                if si2 is None:
                    d2.ins.sync_info = mybir.SyncInfo(on_wait=[w], on_update=[])
                else:
                    si2.on_wait = [w]
        nc.all_engine_barrier()
        assert self.sems is not None
        popped = nc._tile_sem_poison_stack.pop()
        assert popped is self._sem_poison
        nc.clear_and_free_semaphores(list(self.sems.allocated().values()))
        nc.all_engine_barrier()

    tile.TileContext._drain_and_barrier = _drain_and_barrier


_ENGINE_ATTR = {
    "EngineType.SP": "sync",
    "EngineType.PE": "tensor",
    "EngineType.DVE": "vector",
    "EngineType.Activation": "scalar",
    "EngineType.Pool": "gpsimd",
}

# Instruction types whose walrus encodings accept only one sync-wait command.
_SINGLE_WAIT_TYPES = ("InstDMACopy", "InstDrain", "InstTensorLoad", "InstTensorSave")


def _split_excess_waits(nc, max_w=1):
    """Walrus post-pass: some instruction encodings (CTRL, DMA pseudo-ops)
    accept a single sync-wait. Hoist excess waits onto same-engine NOPs
    inserted immediately before the instruction (program order on the engine
    makes waiting earlier always safe)."""
    fn = nc.m.functions[0]
    for bb in fn.blocks:
        insts = list(bb.instructions)
        fixes = []  # (index, inst, excess_waits)
        for idx, inst in enumerate(insts):
            si = inst.sync_info
            if si is not None and si.on_wait and len(si.on_wait) > max_w:
                waits = list(si.on_wait)
                si.on_wait = waits[:max_w]
                fixes.append((idx, inst, waits[max_w:]))
        if not fixes:
            continue
        # create NOPs (they append to the current tail block; pop them off)
        tail_bb = fn.blocks[-1]
        newlist = []
        fix_map = {id(inst): ws for _, inst, ws in fixes}
        for inst in insts:
            ws = fix_map.get(id(inst))
            if ws:
                eng = _ENGINE_ATTR[str(inst.engine)]
                for w in ws:
                    nop = getattr(nc, eng).nop()
                    nop_inst = nop.ins if hasattr(nop, "ins") else nop
                    tail = list(tail_bb.instructions)
                    assert tail and tail[-1] is nop_inst
                    tail_bb.instructions = tail[:-1]
                    nsi = nop_inst.sync_info
                    if nsi is None:
                        nop_inst.sync_info = mybir.SyncInfo(on_wait=[w], on_update=[])
                    else:
                        nsi.on_wait = [w]
                    newlist.append(nop_inst)
            newlist.append(inst)
        bb.instructions = newlist


# ---------------------------------------------------------------------------
_NC_CACHE = {}


def _build_program():
    """Build the single-core Bass/Tile program (same NEFF runs on all 8 cores)."""
    if "nc" in _NC_CACHE:
        return _NC_CACHE["nc"]
    _install_drain_patch()

    f32 = mybir.dt.float32
    bf16 = mybir.dt.bfloat16

    nc = bass.Bass("TRN2", target_bir_lowering=False, debug=False)
    obs_ap = nc.dram_tensor("obs", [BL, S, FA], bf16, kind="ExternalInput").ap()
    wh_ap = nc.dram_tensor("wh", [128, 2 * G], bf16, kind="ExternalInput").ap()
    wi_ap = nc.dram_tensor("wi", [FA, G], bf16, kind="ExternalInput").ap()
    wd_ap = nc.dram_tensor("wd", [128, 2 * D], bf16, kind="ExternalInput").ap()
    bd_ap = nc.dram_tensor("bd", [D, 1], f32, kind="ExternalInput").ap()
    idb_ap = nc.dram_tensor("idb", [128, 128], bf16, kind="ExternalInput").ap()
    idf_ap = nc.dram_tensor("idf", [128, 128], f32, kind="ExternalInput").ap()
    out_ap = nc.dram_tensor("out", [BL, D], f32, kind="ExternalOutput").ap()

    from contextlib import ExitStack

    with tile.TileContext(nc) as tc, ExitStack() as ctx:
        wpool = ctx.enter_context(tc.tile_pool(name="weights", bufs=1))
        xs_pool = ctx.enter_context(tc.tile_pool(name="xs", bufs=4))
        xt_pool = ctx.enter_context(tc.tile_pool(name="xt", bufs=4))
        st_pool = ctx.enter_context(tc.tile_pool(name="state", bufs=3))
        ga_pool = ctx.enter_context(tc.tile_pool(name="gact", bufs=6))
        ps_g = ctx.enter_context(tc.tile_pool(name="psg", bufs=2, space="PSUM"))
        ps_x = ctx.enter_context(tc.tile_pool(name="psx", bufs=1, space="PSUM"))

        # --- weights / constants ------------------------------------------
        wh_sb = wpool.tile([128, 2 * G], bf16, tag="wh")
        nc.sync.dma_start(wh_sb[:], wh_ap[:])
        wi_sb = wpool.tile([FA, G], bf16, tag="wi")
        nc.sync.dma_start(wi_sb[:], wi_ap[:])
        wd_sb = wpool.tile([128, 2 * D], bf16, tag="wd")
        nc.sync.dma_start(wd_sb[:], wd_ap[:])
        bd_sb = wpool.tile([D, 1], f32, tag="bd")
        nc.sync.dma_start(bd_sb[:], bd_ap[:])
        idb_sb = wpool.tile([128, 128], bf16, tag="idb")
        nc.sync.dma_start(idb_sb[:], idb_ap[:])
        idf_sb = wpool.tile([128, 128], f32, tag="idf")
        nc.sync.dma_start(idf_sb[:], idf_ap[:])

        # --- initial state -------------------------------------------------
        hT = st_pool.tile([128, 512], bf16, tag="hT")
        nc.gpsimd.memset(hT[:], 0.0)
        c_prev = st_pool.tile([128, 512], bf16, tag="c")
        nc.gpsimd.memset(c_prev[:], 0.0)

        # --- obs chunk loads ----------------------------------------------
        chunks = {}

        def load_chunk(ci):
            tiles = []
            for bt in range(2):
                t = xs_pool.tile([128, CH * FA], bf16, tag=f"xs{bt}")
                nc.sync.dma_start(
                    t[:].rearrange("p (k f) -> p k f", f=FA),
                    obs_ap[bt * 128:(bt + 1) * 128, ci * CH:(ci + 1) * CH, :],
                )
                tiles.append(t)
            chunks[ci] = tiles

        def get_chunk(ci):
            if ci not in chunks:
                load_chunk(ci)
            return chunks[ci]

        # --- x transposes: steps [s0, s0+TB) -> xT tile [FA, TB*256] ------
        def make_xt(s0):
            ctiles = get_chunk(s0 // CH)
            ps = ps_x.tile([FA, TB * 256], bf16, tag="xp")
            for kk in range(TB):
                off = ((s0 + kk) % CH) * FA
                for bt in range(2):
                    nc.tensor.transpose(
                        ps[:, kk * 256 + bt * 128: kk * 256 + bt * 128 + 128],
                        ctiles[bt][:, off:off + FA],
                        idb_sb[:],
                    )
            xt = xt_pool.tile([FA, TB * 256], bf16, tag="xt")
            nc.vector.tensor_copy(xt[:], ps[:])
            return xt

        load_chunk(0)
        load_chunk(1)
        xts = {0: make_xt(0)}

        # --- main recurrence ----------------------------------------------
        for s in range(S):
            gi = s // TB
            if s % TB == 0:
                if s + TB < S:
                    xts[gi + 1] = make_xt(s + TB)
                ci_pref = (s + TB) // CH + 1
                if ci_pref * CH < S and ci_pref not in chunks:
                    load_chunk(ci_pref)
                if gi - 1 in xts:
                    del xts[gi - 1]

            xt = xts[gi]
            xoff = (s % TB) * 256

            g_lo = ps_g.tile([128, G], f32, tag="g")
            g_hi = ps_g.tile([128, G], f32, tag="g")
            g_ht = (g_lo, g_hi)
            for j in range(8):
                out_j = g_ht[j // 4][:, (j % 4) * 256:(j % 4 + 1) * 256]
                nc.tensor.matmul(
                    out_j, wi_sb[:, j * 128:(j + 1) * 128],
                    xt[:, xoff:xoff + 256], start=True, stop=False,
                )
                nc.tensor.matmul(
                    out_j, wh_sb[:, j * 128:(j + 1) * 128],
                    hT[:, 0:256], start=False, stop=False,
                )
                nc.tensor.matmul(
                    out_j, wh_sb[:, G + j * 128:G + j * 128 + 128],
                    hT[:, 256:512], start=False, stop=True,
                )

            # activations: gate cols per h-tile are [i | f | o | g] * 256
            sg0 = ga_pool.tile([128, 768], bf16, tag="sg")
            nc.scalar.activation(sg0[:], g_ht[0][:, 0:768], AFT.Sigmoid)
            tg0 = ga_pool.tile([128, 256], bf16, tag="tg")
            nc.scalar.activation(tg0[:], g_ht[0][:, 768:1024], AFT.Tanh)
            sg1 = ga_pool.tile([128, 768], bf16, tag="sg")
            nc.scalar.activation(sg1[:], g_ht[1][:, 0:768], AFT.Sigmoid)
            tg1 = ga_pool.tile([128, 256], bf16, tag="tg")
            nc.scalar.activation(tg1[:], g_ht[1][:, 768:1024], AFT.Tanh)

            c_new = st_pool.tile([128, 512], bf16, tag="c")
            h_new = st_pool.tile([128, 512], bf16, tag="hT")
            hmul = []
            for ht, (sg, tg) in enumerate(((sg0, tg0), (sg1, tg1))):
                f_ = sg[:, 256:512]
                i_ = sg[:, 0:256]
                o_ = sg[:, 512:768]
                cs = slice(ht * 256, (ht + 1) * 256)
                t1 = ga_pool.tile([128, 256], bf16, tag="t1")
                nc.vector.tensor_mul(t1[:], f_, c_prev[:, cs])
                t2 = ga_pool.tile([128, 256], bf16, tag="t2")
                nc.vector.tensor_mul(t2[:], i_, tg[:])
                nc.vector.tensor_add(c_new[:, cs], t1[:], t2[:])
                tc_t = ga_pool.tile([128, 256], bf16, tag="tc")
                nc.scalar.activation(tc_t[:], c_new[:, cs], AFT.Tanh)
                hmul.append((o_, tc_t, cs))
            for o_, tc_t, cs in hmul:
                nc.vector.tensor_mul(h_new[:, cs], o_, tc_t[:])

            c_prev = c_new
            hT = h_new

        # --- final dense + relu + output ----------------------------------
        with tc.tile_pool(name="fin", bufs=1) as fin, \
             tc.tile_pool(name="psf", bufs=1, space="PSUM") as psf:
            ot_ps = psf.tile([D, 256], mybir.dt.float32, tag="ot")
            nc.tensor.matmul(ot_ps[:], wd_sb[:, 0:D], hT[:, 0:256],
                             start=True, stop=False)
            nc.tensor.matmul(ot_ps[:], wd_sb[:, D:2 * D], hT[:, 256:512],
                             start=False, stop=True)
            ot_sb = fin.tile([D, 256], mybir.dt.float32, tag="ot_sb")
            nc.scalar.activation(ot_sb[:], ot_ps[:], AFT.Relu, bias=bd_sb[:])
            # transpose [d, b] -> [b, d] on PE, then contiguous DMA out
            otp = psf.tile([128, 256], mybir.dt.float32, tag="otp")
            nc.tensor.transpose(otp[:, 0:128], ot_sb[:, 0:128], idf_sb[:])
            nc.tensor.transpose(otp[:, 128:256], ot_sb[:, 128:256], idf_sb[:])
            fin2 = fin.tile([128, 256], mybir.dt.float32, tag="fin2")
            nc.vector.tensor_copy(fin2[:], otp[:])
            nc.sync.dma_start(
                out_ap.rearrange("(t p) d -> p t d", p=128),
                fin2[:].rearrange("p (t d) -> p t d", d=D),
            )

    _split_excess_waits(nc)
    _NC_CACHE["nc"] = nc
    return nc


# ---------------------------------------------------------------------------
def _host_prep(observations, Wi, Wh, bh, Wd, bd):
    """Permute / augment / cast weights and obs on the host."""
    # gate order [i, f, o, g] per h-tile; orig col bases: i=0, f=256, g=512, o=768
    perm = []
    for ht in range(2):
        for base in (0, 256, 768, 512):
            perm.extend(range(base + ht * 128, base + ht * 128 + 128))
    perm = np.asarray(perm)

    obs_aug = np.empty((B, S, FA), dtype=BF16)
    obs_aug[:, :, :F] = observations.astype(BF16)
    obs_aug[:, :, F] = np.asarray(1.0, dtype=BF16)

    wh_h = np.ascontiguousarray(
        Wh[:, perm].reshape(2, 128, G).transpose(1, 0, 2).reshape(128, 2 * G)
    ).astype(BF16)
    wi_h = np.ascontiguousarray(
        np.concatenate([Wi, bh[None, :]], axis=0)[:, perm]
    ).astype(BF16)
    wd_h = np.ascontiguousarray(
        Wd.reshape(2, 128, D).transpose(1, 0, 2).reshape(128, 2 * D)
    ).astype(BF16)
    bd_h = np.ascontiguousarray(bd.reshape(D, 1)).astype(np.float32)
    idb = np.eye(128, dtype=BF16)
    idf = np.eye(128, dtype=np.float32)
    return obs_aug, wh_h, wi_h, wd_h, bd_h, idb, idf


TRACE = False
LAST_RESULT = None


def kernel(observations, Wi, Wh, bh, Wd, bd):
    global LAST_RESULT
    observations = np.asarray(observations, dtype=np.float32)
    Wi = np.asarray(Wi, dtype=np.float32)
    Wh = np.asarray(Wh, dtype=np.float32)
    bh = np.asarray(bh, dtype=np.float32)
    Wd = np.asarray(Wd, dtype=np.float32)
    bd = np.asarray(bd, dtype=np.float32)

    obs_aug, wh_h, wi_h, wd_h, bd_h, idb, idf = _host_prep(
        observations, Wi, Wh, bh, Wd, bd
    )

    nc = _build_program()
    in_maps = []
    for c in range(NCORES):
        in_maps.append({
            "obs": np.ascontiguousarray(obs_aug[c * BL:(c + 1) * BL]),
            "wh": wh_h,
            "wi": wi_h,
            "wd": wd_h,
            "bd": bd_h,
            "idb": idb,
            "idf": idf,
        })
    res = run_bass_kernel_spmd(
        nc, in_maps, core_ids=list(range(NCORES)), trace=TRACE
    )
    LAST_RESULT = res
    out = np.concatenate([r["out"] for r in res.results], axis=0)
    return out.astype(np.float32)
